# revision 20
# baseline (speedup 1.0000x reference)
"""Causal single-head attention (B=4, S=4096, E=1024, H=64) on 8 trn2 cores.

Sharding: core j handles batch j//2, query parity p=j%2 (256-row query
blocks interleaved by parity). Host permutes the batch's rows by 256-blocks
(pos 2m <- block 2m+p, pos 2m+1 <- block 2m+1-p) so every core runs the
same static program: query slot k = permuted rows [512k, 512k+256), its
causal kv set = permuted rows [0, 512k+512) with a fixed triangular mask on
the first half of the diagonal 512-chunk and a per-core constant mask on the
second half.

On-device dataflow per core:
  - emb arrives host-permuted, bf16, host-transposed to [E, S], stored
    chunk-major [NS, P, NE*SC] so each 512-pos chunk is ONE dma with 8KB
    contiguous per-partition runs. Only the emb chunks + wqk go through
    HWDGE (sync queue) -- chunk 0 as two halves so projection starts on the
    first half early; all other constants and the output stores use the
    gpsimd SWDGE path so nothing queues ahead of chunk 0.
  - QK.T = [WqT|WkT] @ embT (PSUM f32, PE bf16) -> ktsb/qtsb (bf16).
  - V natural [kv, 64] via embT-chunk-stationary matmuls; a ones column pair
    (cols 64:66) makes the PV matmul also produce the softmax denominator.
  - scores.T[kv, q] = K.T-chunk.T @ Q.T-slot; the sibling-parity half-chunk
    mask is folded into the matmul as a 65th contraction row (ktsb row 64 =
    1.0 input, qtsb row 64 = per-core constant input); diagonal triangle via
    DVE adds. exp via ScalarE (scale=1/8).
  - PV flipped: stationary u [kv, q-half], moving V [kv, 66] -> O [q, 66]
    accumulates directly in natural layout (stream 66 cols/tile vs 256), so
    no PE transpose; q-rows scaled by reciprocal of denominator col 64.
"""

import sys

sys.path.insert(0, "/opt/trn_rl_repo")

import numpy as np
import ml_dtypes

import concourse.bass as bass
import concourse.mybir as mybir
import concourse.tile as tile
from concourse import bacc
from concourse.bass_utils import run_bass_kernel_spmd

B, S, E, H = 4, 4096, 1024, 64
P = 128
NE = E // P  # 8 e-chunks
SC = 512  # s-chunk (proj streaming granularity)
NS = S // SC  # 8 s-chunks
QB = 256  # query block (slot) size
NSLOT = S // (2 * QB)  # 8 slots per core
NKV = S // P  # 32 kv tiles
NT = SC // P  # 4 kv tiles per chunk
NEG = -10000.0
F32 = mybir.dt.float32
F32R = mybir.dt.float32r
BF16 = mybir.dt.bfloat16

_CACHE = {}


def _build_program():
    nc = bacc.Bacc("TRN2", target_bir_lowering=False, debug=False, num_devices=8)
    emb = nc.declare_dram_parameter("emb", [NS, P, NE * SC], BF16, isOutput=False)
    wqk = nc.declare_dram_parameter("wqk", [P, NE * P], BF16, isOutput=False)
    wv = nc.declare_dram_parameter("wv", [P, NE * H], BF16, isOutput=False)
    trimask = nc.declare_dram_parameter("trimask", [P, 2 * QB], F32, isOutput=False)
    m2row = nc.declare_dram_parameter("m2row", [1, NSLOT * QB], BF16, isOutput=False)
    krow1 = nc.declare_dram_parameter("krow1", [1, S], BF16, isOutput=False)
    out = nc.declare_dram_parameter("out", [2 * NSLOT, P, H], F32, isOutput=True)

    with tile.TileContext(nc) as tc:
        with (
            tc.tile_pool(name="persist", bufs=1) as pers,
            tc.tile_pool(name="embt", bufs=4) as embtp,
            tc.tile_pool(name="upool", bufs=4) as upool,
            tc.tile_pool(name="osmall", bufs=4) as osmall,
            tc.tile_pool(name="pp", bufs=2, space="PSUM") as pp,
            tc.tile_pool(name="scp", bufs=2, space="PSUM") as scp,
            tc.tile_pool(name="oaccp", bufs=2, space="PSUM") as oaccp,
        ):
            # ---- critical-path loads on HWDGE (sync queue): wqk then emb.
            # First the e0 slices of wqk and chunk 0 (160KB) so the first
            # projection matmul starts ~3us earlier than a monolithic load.
            wqk_sb = pers.tile([P, NE * P], BF16, tag="wqk")
            nc.sync.dma_start(wqk_sb[:, 0:P], wqk[:, 0:P])
            h0 = embtp.tile([P, 2 * SC], BF16, tag="embt", name="h0")
            nc.sync.dma_start(out=h0[:, 0:SC], in_=emb[0][:, 0:SC])
            nc.sync.dma_start(wqk_sb[:, P : NE * P], wqk[:, P : NE * P])
            nc.sync.dma_start(out=h0[:, SC : 2 * SC], in_=emb[0][:, SC : 2 * SC])
            h1 = embtp.tile([P, (NE - 2) * SC], BF16, tag="embt", name="h1")
            nc.sync.dma_start(out=h1[:, 0 : 3 * SC], in_=emb[0][:, 2 * SC : 5 * SC])
            nc.sync.dma_start(
                out=h1[:, 3 * SC : 6 * SC], in_=emb[0][:, 5 * SC : NE * SC]
            )

            def load_chunk(m):
                # two half DMAs so projection starts on e<4 ~1.5us earlier
                # (subtile deps let e<4 readers run before the second half)
                et = embtp.tile([P, NE * SC], BF16, tag="embt")
                nc.sync.dma_start(out=et[:, 0 : 4 * SC], in_=emb[m][:, 0 : 4 * SC])
                nc.sync.dma_start(
                    out=et[:, 4 * SC : NE * SC], in_=emb[m][:, 4 * SC : NE * SC]
                )
                return lambda e: et[:, e * SC : (e + 1) * SC]

            def chunk0(e):
                if e < 2:
                    return h0[:, e * SC : (e + 1) * SC]
                return h1[:, (e - 2) * SC : (e - 1) * SC]

            ets_q = [chunk0, load_chunk(1), load_chunk(2)]

            # ---- everything else via gpsimd SWDGE (off the HWDGE path) ----
            wv_sb = pers.tile([P, NE * H], BF16, tag="wv")
            nc.gpsimd.dma_start(out=wv_sb[:], in_=wv[:])
            tri_sb = pers.tile([P, 2, QB], F32, tag="tri")
            nc.gpsimd.dma_start(out=tri_sb[:], in_=trimask[:])
            qtsb = pers.tile([H + 1, NSLOT * QB], BF16, tag="qt")
            nc.gpsimd.dma_start(out=qtsb[H : H + 1, :], in_=m2row[:])
            ktsb = pers.tile([H + 1, S], BF16, tag="kt")
            nc.gpsimd.dma_start(out=ktsb[H : H + 1, :], in_=krow1[:])
            vsb = pers.tile([P, NKV, 72], BF16, tag="v")
            nc.vector.memset(vsb[:, :, H : H + 2], 1.0)

            # activation-table warm-up (Exp) on the earliest-landing tile
            warm = pers.tile([P, 1], F32, tag="warm")
            nc.scalar.activation(
                warm[:], wqk_sb[:, 0:1], mybir.ActivationFunctionType.Exp, scale=0.125
            )

            # PE p-state warm-up: dummy matmuls on a memset tile keep the PE
            # continuously busy through the initial DMA wait so the first
            # real projection runs at full clock instead of the ramp rate.
            wmt = pers.tile([P, SC], BF16, tag="wmt")
            nc.vector.memset(wmt[:], 0.0)
            wps = scp.tile([P, 4, QB], F32, tag="sc", name="wps")
            for _ in range(8):
                nc.tensor.matmul(
                    wps[:, 0:2, :], wmt[:, 0:P], wmt[:], start=True, stop=True
                )

            def proj_chunk(m, geta):
                # e-major interleave of QK and V so projection tracks the
                # chunk DMA arrival slice by slice (no stall on the second
                # half-chunk blocking V work behind it)
                qk = pp.tile([P, SC], F32, tag="pp")
                vn4 = pp.tile([P, NT, H], F32, tag="pp")
                for e in range(NE):
                    nc.tensor.matmul(
                        qk[:],
                        wqk_sb[:, e * P : (e + 1) * P],
                        geta(e),
                        start=(e == 0),
                        stop=(e == NE - 1),
                    )
                    for t in range(NT):
                        # one start/stop for the whole vn4 bank (lazy zero
                        # covers all four t-regions; per-t starts would
                        # re-mark siblings' partials as pending-zero)
                        nc.tensor.matmul(
                            vn4[:, t, :],
                            geta(e)[:, t * P : (t + 1) * P],
                            wv_sb[:, e * H : (e + 1) * H],
                            start=(e == 0 and t == 0),
                            stop=(e == NE - 1 and t == NT - 1),
                        )
                nc.vector.tensor_copy(qtsb[0:H, m * QB : (m + 1) * QB], qk[0:H, 0:QB])
                nc.vector.tensor_copy(ktsb[0:H, m * SC : (m + 1) * SC], qk[H:P, :])
                nc.vector.tensor_copy(vsb[:, m * NT : (m + 1) * NT, 0:H], vn4[:])

            def scores_group(k, g, split_exp=False):
                sc_t = scp.tile([P, 4, QB], F32, tag="sc")
                for j in range(4):
                    tkv = 4 * g + j
                    rows = H + 1 if (g == k and j >= 2) else H
                    nc.tensor.matmul(
                        sc_t[:, j, :],
                        ktsb[0:rows, tkv * P : (tkv + 1) * P],
                        qtsb[0:rows, k * QB : (k + 1) * QB],
                        start=True,
                        stop=True,
                    )
                if g == k:  # diagonal group: triangular causal masks on j=0,1
                    # both mask tiles are adjacent -> one 512-wide DVE add
                    nc.vector.tensor_add(sc_t[:, 0:2, :], sc_t[:, 0:2, :], tri_sb[:])
                u = upool.tile([P, 4, QB], BF16, tag="u")
                if split_exp:
                    # kernel-final group: exp in halves so PV starts after
                    # the first half, shortening the tail's serial chain
                    for half in range(2):
                        nc.scalar.activation(
                            u[:, 2 * half : 2 * half + 2, :],
                            sc_t[:, 2 * half : 2 * half + 2, :],
                            mybir.ActivationFunctionType.Exp,
                            scale=0.125,
                        )
                else:
                    nc.scalar.activation(
                        u[:], sc_t[:], mybir.ActivationFunctionType.Exp, scale=0.125
                    )
                return u

            def pv_group(k, g, ot, u, first, last):
                # flipped PV: stationary = u q-half [128kv, 128q], moving =
                # V natural [128kv, 66] -> O[q, 66] accumulates in PSUM in
                # the output's natural layout (66-col stream vs 256)
                # one PSUM bank holds both q-half accumulators: start marks
                # the whole 2KB zero-region pending (lazy zero-on-write), so
                # only the very first matmul starts and only the very last
                # stops -- both 264B regions inherit the pending-zero.
                for j in range(4):
                    tkv = 4 * g + j
                    for h2 in range(2):
                        nc.tensor.matmul(
                            ot[:, h2, :],
                            u[:, j, h2 * P : (h2 + 1) * P],
                            vsb[:, tkv, 0 : H + 2],
                            start=(first and j == 0 and h2 == 0),
                            stop=(last and j == 3 and h2 == 1),
                        )

            def finalize_slot(k, ot, tail=False):
                for h2 in range(2):
                    rec = osmall.tile([P, 1], F32, tag="rec")
                    nc.vector.reciprocal(rec[:], ot[:, h2, H : H + 1])
                    o_t = osmall.tile([P, H], F32, tag="o")
                    nc.vector.tensor_scalar_mul(o_t[:], ot[:, h2, 0:H], rec[:])
                    if tail:
                        # tail stores: the idle HWDGE queues beat the gpsimd
                        # SWDGE launch+prep latency
                        eng = nc.scalar if h2 == 0 else nc.sync
                        eng.dma_start(out=out[2 * k + h2], in_=o_t[:])
                    else:
                        nc.gpsimd.dma_start(out=out[2 * k + h2], in_=o_t[:])

            proj_chunk(0, ets_q[0])
            pend = None
            for k in range(NSLOT):
                if k + 3 < NS:
                    ets_q.append(load_chunk(k + 3))
                # diagonal group first: its DVE mask adds + exp leave the
                # slot's (and kernel's) critical tail
                glist = [k] + list(range(k))
                u = scores_group(k, glist[0])
                ot = oaccp.tile([P, 2, H + 2], F32, tag="ot")
                if pend is not None:
                    finalize_slot(pend[0], pend[1])
                    pend = None
                for i, g in enumerate(glist):
                    if i + 1 < len(glist):
                        u_next = scores_group(
                            k,
                            glist[i + 1],
                            split_exp=(k == NSLOT - 1 and i + 2 == len(glist)),
                        )
                    else:
                        u_next = None
                        if k + 1 < NS:
                            proj_chunk(k + 1, ets_q[k + 1])
                    pv_group(k, g, ot, u, first=(i == 0), last=(i + 1 == len(glist)))
                    u = u_next
                pend = (k, ot)
            finalize_slot(pend[0], pend[1], tail=True)
    nc.compile()
    return nc


def _host_inputs(embeddings, W_Q, W_K, W_V):
    """Build the 8 per-core input maps."""
    wqk = np.empty((NE, P, P), np.float32)
    wv = np.empty((NE, P, H), np.float32)
    for c in range(NE):
        wqk[c, :, 0:H] = W_Q[:, c * P : (c + 1) * P].T
        wqk[c, :, H:P] = W_K[:, c * P : (c + 1) * P].T
        wv[c] = W_V[:, c * P : (c + 1) * P].T
    # device layout: [P, NE*P] / [P, NE*H] (e-chunk-major along free dim)
    wqk = np.ascontiguousarray(wqk.transpose(1, 0, 2).reshape(P, NE * P)).astype(
        ml_dtypes.bfloat16
    )
    wv = np.ascontiguousarray(wv.transpose(1, 0, 2).reshape(P, NE * H)).astype(
        ml_dtypes.bfloat16
    )

    ki = np.arange(P)[:, None]
    qj = np.arange(QB)[None, :]
    trimask = np.zeros((P, 2 * QB), np.float32)
    trimask[:, 0:QB] = np.where(qj >= ki, 0.0, NEG)
    trimask[:, QB : 2 * QB] = np.where(qj >= ki + P, 0.0, NEG)
    krow1 = np.ones((1, S), np.float32).astype(ml_dtypes.bfloat16)

    in_maps = []
    for j in range(8):
        b, p = j // 2, j % 2
        eb = embeddings[b].reshape(S // QB, QB, E)
        order = np.empty(S // QB, np.int64)
        for m in range(S // (2 * QB)):
            order[2 * m] = 2 * m + p
            order[2 * m + 1] = 2 * m + 1 - p
        # [NS, P, NE*SC]: chunk-major, within chunk partition-major, e-chunk
        # then position: emb[m, p_, e*SC+s] = embT[e*P + p_, m*SC + s]
        embp = np.ascontiguousarray(
            eb[order]
            .reshape(S, E)
            .astype(ml_dtypes.bfloat16)
            .T.reshape(NE, P, NS, SC)
            .transpose(2, 1, 0, 3)
            .reshape(NS, P, NE * SC)
        )
        m2 = np.full((1, NSLOT * QB), NEG if p == 0 else 0.0, np.float32).astype(
            ml_dtypes.bfloat16
        )
        in_maps.append(
            {
                "emb": embp,
                "wqk": wqk,
                "wv": wv,
                "trimask": trimask,
                "m2row": m2,
                "krow1": krow1,
            }
        )
    return in_maps


def _assemble(results):
    out = np.empty((B, S, H), np.float32)
    for j in range(8):
        b, p = j // 2, j % 2
        o = results[j]["out"]  # [16, 128, 64]
        for k in range(NSLOT):
            g0 = (2 * k + p) * QB
            out[b, g0 : g0 + P] = o[2 * k]
            out[b, g0 + P : g0 + 2 * P] = o[2 * k + 1]
    return out


def kernel(embeddings, W_Q, W_K, W_V, _trace=False, _tmpdir=None):
    if "nc" not in _CACHE:
        _CACHE["nc"] = _build_program()
    nc = _CACHE["nc"]
    in_maps = _host_inputs(
        np.asarray(embeddings), np.asarray(W_Q), np.asarray(W_K), np.asarray(W_V)
    )
    res = run_bass_kernel_spmd(
        nc, in_maps, list(range(8)), trace=_trace, tmpdir=_tmpdir
    )
    out = _assemble(res.results)
    if _trace:
        return out, res
    return out


if __name__ == "__main__":
    rng = np.random.default_rng(0)
    emb = rng.standard_normal((B, S, E), dtype=np.float32)
    wq = rng.uniform(-0.07, 0.07, (H, E)).astype(np.float32)
    wk = rng.uniform(-0.07, 0.07, (H, E)).astype(np.float32)
    wv_ = rng.uniform(-0.07, 0.07, (H, E)).astype(np.float32)
    o = kernel(emb, wq, wk, wv_)
    print("ok", o.shape, o.dtype)



# revision 23
# speedup vs baseline: 1.1890x; 1.1890x over previous
"""Causal single-head attention (B=4, S=4096, E=1024, H=64) on 8 trn2 cores.

Sharding: core j handles batch j//2, query parity p=j%2 (256-row query
blocks interleaved by parity). Host permutes the batch's rows by 256-blocks
(pos 2m <- block 2m+p, pos 2m+1 <- block 2m+1-p) so every core runs the
same static program: query slot k = permuted rows [512k, 512k+256), its
causal kv set = permuted rows [0, 512k+512) with a fixed triangular mask on
the first half of the diagonal 512-chunk and a per-core constant mask on the
second half.

On-device dataflow per core:
  - emb arrives host-permuted, bf16, host-transposed to [E, S], stored
    chunk-major [NS, P, NE*SC] so each 512-pos chunk is ONE dma with 8KB
    contiguous per-partition runs. Only the emb chunks + wqk go through
    HWDGE (sync queue) -- chunk 0 as two halves so projection starts on the
    first half early; all other constants and the output stores use the
    gpsimd SWDGE path so nothing queues ahead of chunk 0.
  - QK.T = [WqT|WkT] @ embT (PSUM f32, PE bf16) -> ktsb/qtsb (bf16).
  - V natural [kv, 64] via embT-chunk-stationary matmuls; a ones column pair
    (cols 64:66) makes the PV matmul also produce the softmax denominator.
  - scores.T[kv, q] = K.T-chunk.T @ Q.T-slot; the sibling-parity half-chunk
    mask is folded into the matmul as a 65th contraction row (ktsb row 64 =
    1.0 input, qtsb row 64 = per-core constant input); diagonal triangle via
    DVE adds. exp via ScalarE (scale=1/8).
  - PV flipped: stationary u [kv, q-half], moving V [kv, 66] -> O [q, 66]
    accumulates directly in natural layout (stream 66 cols/tile vs 256), so
    no PE transpose; q-rows scaled by reciprocal of denominator col 64.
"""

import sys

sys.path.insert(0, "/opt/trn_rl_repo")

import numpy as np
import ml_dtypes

import concourse.bass as bass
import concourse.mybir as mybir
import concourse.tile as tile
from concourse import bacc
from concourse.bass_utils import run_bass_kernel_spmd

B, S, E, H = 4, 4096, 1024, 64
P = 128
NE = E // P  # 8 e-chunks
SC = 512  # s-chunk (proj streaming granularity)
NS = S // SC  # 8 s-chunks
QB = 256  # query block (slot) size
NSLOT = S // (2 * QB)  # 8 slots per core
NKV = S // P  # 32 kv tiles
NT = SC // P  # 4 kv tiles per chunk
NEG = -10000.0
F32 = mybir.dt.float32
F32R = mybir.dt.float32r
BF16 = mybir.dt.bfloat16

_CACHE = {}


def _build_program():
    nc = bacc.Bacc("TRN2", target_bir_lowering=False, debug=False, num_devices=8)
    emb = nc.declare_dram_parameter("emb", [NS, P, NE * SC], BF16, isOutput=False)
    wqk = nc.declare_dram_parameter("wqk", [P, NE * P], BF16, isOutput=False)
    wv = nc.declare_dram_parameter("wv", [P, NE * H], BF16, isOutput=False)
    trimask = nc.declare_dram_parameter("trimask", [P, 2 * QB], F32, isOutput=False)
    m2row = nc.declare_dram_parameter("m2row", [1, NSLOT * QB], BF16, isOutput=False)
    krow1 = nc.declare_dram_parameter("krow1", [1, S], BF16, isOutput=False)
    out = nc.declare_dram_parameter("out", [2 * NSLOT, P, H], F32, isOutput=True)

    with tile.TileContext(nc) as tc:
        with (
            tc.tile_pool(name="persist", bufs=1) as pers,
            tc.tile_pool(name="embt", bufs=4) as embtp,
            tc.tile_pool(name="upool", bufs=4) as upool,
            tc.tile_pool(name="osmall", bufs=4) as osmall,
            tc.tile_pool(name="pp", bufs=2, space="PSUM") as pp,
            tc.tile_pool(name="scp", bufs=2, space="PSUM") as scp,
            tc.tile_pool(name="oaccp", bufs=2, space="PSUM") as oaccp,
        ):
            # ---- critical-path loads on HWDGE (sync queue): wqk then emb.
            # The e0 slices of wqk and chunk 0 (160KB) go first so the first
            # projection matmul starts ~2.5us earlier than monolithic loads.
            wqk_sb = pers.tile([P, NE * P], BF16, tag="wqk")
            nc.sync.dma_start(wqk_sb[:, 0:P], wqk[:, 0:P])
            h0 = embtp.tile([P, 2 * SC], BF16, tag="embt", name="h0")
            nc.sync.dma_start(out=h0[:, 0:SC], in_=emb[0][:, 0:SC])
            nc.sync.dma_start(wqk_sb[:, P : NE * P], wqk[:, P : NE * P])
            nc.sync.dma_start(out=h0[:, SC : 2 * SC], in_=emb[0][:, SC : 2 * SC])
            h1 = embtp.tile([P, (NE - 2) * SC], BF16, tag="embt", name="h1")
            nc.sync.dma_start(out=h1[:], in_=emb[0][:, 2 * SC : NE * SC])

            def load_chunk(m):
                # two half DMAs so projection starts on e<4 ~1.5us earlier
                # (subtile deps let e<4 readers run before the second half)
                et = embtp.tile([P, NE * SC], BF16, tag="embt")
                nc.sync.dma_start(out=et[:, 0 : 4 * SC], in_=emb[m][:, 0 : 4 * SC])
                nc.sync.dma_start(
                    out=et[:, 4 * SC : NE * SC], in_=emb[m][:, 4 * SC : NE * SC]
                )
                return lambda e: et[:, e * SC : (e + 1) * SC]

            def chunk0(e):
                if e < 2:
                    return h0[:, e * SC : (e + 1) * SC]
                return h1[:, (e - 2) * SC : (e - 1) * SC]

            ets_q = [chunk0, load_chunk(1), load_chunk(2)]

            # ---- everything else via gpsimd SWDGE (off the HWDGE path) ----
            wv_sb = pers.tile([P, NE * H], BF16, tag="wv")
            nc.gpsimd.dma_start(out=wv_sb[:], in_=wv[:])
            tri_sb = pers.tile([P, 2, QB], F32, tag="tri")
            nc.gpsimd.dma_start(out=tri_sb[:], in_=trimask[:])
            qtsb = pers.tile([H + 1, NSLOT * QB], BF16, tag="qt")
            nc.gpsimd.dma_start(out=qtsb[H : H + 1, :], in_=m2row[:])
            ktsb = pers.tile([H + 1, S], BF16, tag="kt")
            nc.gpsimd.dma_start(out=ktsb[H : H + 1, :], in_=krow1[:])
            vsb = pers.tile([P, NKV, 72], BF16, tag="v")
            nc.vector.memset(vsb[:, :, H : H + 2], 1.0)

            # activation-table warm-up (Exp) on the earliest-landing tile
            warm = pers.tile([P, 1], F32, tag="warm")
            nc.scalar.activation(
                warm[:], wqk_sb[:, 0:1], mybir.ActivationFunctionType.Exp, scale=0.125
            )

            # PE p-state warm-up: dummy matmuls on a memset tile keep the PE
            # continuously busy through the initial DMA wait so the first
            # real projections run at full clock instead of the ramp rate.
            wmt = pers.tile([P, SC], BF16, tag="wmt")
            nc.vector.memset(wmt[:], 0.0)
            wps = scp.tile([P, 4, QB], F32, tag="sc", name="wps")
            for _ in range(8):
                nc.tensor.matmul(
                    wps[:, 0:2, :], wmt[:, 0:P], wmt[:], start=True, stop=True
                )

            def proj_chunk(m, geta):
                qk = pp.tile([P, SC], F32, tag="pp")
                for e in range(NE):
                    nc.tensor.matmul(
                        qk[:],
                        wqk_sb[:, e * P : (e + 1) * P],
                        geta(e),
                        start=(e == 0),
                        stop=(e == NE - 1),
                    )
                nc.vector.tensor_copy(qtsb[0:H, m * QB : (m + 1) * QB], qk[0:H, 0:QB])
                nc.vector.tensor_copy(ktsb[0:H, m * SC : (m + 1) * SC], qk[H:P, :])
                vn4 = pp.tile([P, NT, H], F32, tag="pp")
                for t in range(NT):
                    for e in range(NE):
                        nc.tensor.matmul(
                            vn4[:, t, :],
                            geta(e)[:, t * P : (t + 1) * P],
                            wv_sb[:, e * H : (e + 1) * H],
                            start=(e == 0),
                            stop=(e == NE - 1),
                        )
                nc.vector.tensor_copy(vsb[:, m * NT : (m + 1) * NT, 0:H], vn4[:])

            def scores_group(k, g, split_exp=False):
                sc_t = scp.tile([P, 4, QB], F32, tag="sc")
                for j in range(4):
                    tkv = 4 * g + j
                    rows = H + 1 if (g == k and j >= 2) else H
                    nc.tensor.matmul(
                        sc_t[:, j, :],
                        ktsb[0:rows, tkv * P : (tkv + 1) * P],
                        qtsb[0:rows, k * QB : (k + 1) * QB],
                        start=True,
                        stop=True,
                    )
                if g == k:  # diagonal group: triangular causal masks on j=0,1
                    # both mask tiles are adjacent -> one 512-wide DVE add
                    nc.vector.tensor_add(sc_t[:, 0:2, :], sc_t[:, 0:2, :], tri_sb[:])
                u = upool.tile([P, 4, QB], BF16, tag="u")
                if split_exp:
                    # kernel-final group: exp in halves so PV starts after
                    # the first half, shortening the tail's serial chain
                    for half in range(2):
                        nc.scalar.activation(
                            u[:, 2 * half : 2 * half + 2, :],
                            sc_t[:, 2 * half : 2 * half + 2, :],
                            mybir.ActivationFunctionType.Exp,
                            scale=0.125,
                        )
                else:
                    nc.scalar.activation(
                        u[:], sc_t[:], mybir.ActivationFunctionType.Exp, scale=0.125
                    )
                return u

            def pv_group(k, g, ot, u, first, last):
                # flipped PV: stationary = u q-half [128kv, 128q], moving =
                # V natural [128kv, 66] -> O[q, 66] accumulates in PSUM in
                # the output's natural layout (66-col stream vs 256)
                # one PSUM bank holds both q-half accumulators: start marks
                # the whole 2KB zero-region pending (lazy zero-on-write), so
                # only the very first matmul starts and only the very last
                # stops -- both 264B regions inherit the pending-zero.
                for j in range(4):
                    tkv = 4 * g + j
                    for h2 in range(2):
                        nc.tensor.matmul(
                            ot[:, h2, :],
                            u[:, j, h2 * P : (h2 + 1) * P],
                            vsb[:, tkv, 0 : H + 2],
                            start=(first and j == 0 and h2 == 0),
                            stop=(last and j == 3 and h2 == 1),
                        )

            def finalize_slot(k, ot, tail=False):
                for h2 in range(2):
                    rec = osmall.tile([P, 1], F32, tag="rec")
                    nc.vector.reciprocal(rec[:], ot[:, h2, H : H + 1])
                    o_t = osmall.tile([P, H], F32, tag="o")
                    nc.vector.tensor_scalar_mul(o_t[:], ot[:, h2, 0:H], rec[:])
                    if tail:
                        # tail stores: the idle HWDGE queues beat the gpsimd
                        # SWDGE launch+prep latency
                        eng = nc.scalar if h2 == 0 else nc.sync
                        eng.dma_start(out=out[2 * k + h2], in_=o_t[:])
                    else:
                        nc.gpsimd.dma_start(out=out[2 * k + h2], in_=o_t[:])

            proj_chunk(0, ets_q[0])
            pend = None
            for k in range(NSLOT):
                if k + 3 < NS:
                    ets_q.append(load_chunk(k + 3))
                # diagonal group first: its DVE mask adds + exp leave the
                # slot's (and kernel's) critical tail
                glist = [k] + list(range(k))
                u = scores_group(k, glist[0])
                ot = oaccp.tile([P, 2, H + 2], F32, tag="ot")
                if pend is not None:
                    finalize_slot(pend[0], pend[1])
                    pend = None
                for i, g in enumerate(glist):
                    if i + 1 < len(glist):
                        u_next = scores_group(
                            k,
                            glist[i + 1],
                            split_exp=(k == NSLOT - 1 and i + 2 == len(glist)),
                        )
                    else:
                        u_next = None
                        if k + 1 < NS:
                            proj_chunk(k + 1, ets_q[k + 1])
                    pv_group(k, g, ot, u, first=(i == 0), last=(i + 1 == len(glist)))
                    u = u_next
                pend = (k, ot)
            finalize_slot(pend[0], pend[1], tail=True)
    nc.compile()
    return nc


def _host_inputs(embeddings, W_Q, W_K, W_V):
    """Build the 8 per-core input maps."""
    wqk = np.empty((NE, P, P), np.float32)
    wv = np.empty((NE, P, H), np.float32)
    for c in range(NE):
        wqk[c, :, 0:H] = W_Q[:, c * P : (c + 1) * P].T
        wqk[c, :, H:P] = W_K[:, c * P : (c + 1) * P].T
        wv[c] = W_V[:, c * P : (c + 1) * P].T
    # device layout: [P, NE*P] / [P, NE*H] (e-chunk-major along free dim)
    wqk = np.ascontiguousarray(wqk.transpose(1, 0, 2).reshape(P, NE * P)).astype(
        ml_dtypes.bfloat16
    )
    wv = np.ascontiguousarray(wv.transpose(1, 0, 2).reshape(P, NE * H)).astype(
        ml_dtypes.bfloat16
    )

    ki = np.arange(P)[:, None]
    qj = np.arange(QB)[None, :]
    trimask = np.zeros((P, 2 * QB), np.float32)
    trimask[:, 0:QB] = np.where(qj >= ki, 0.0, NEG)
    trimask[:, QB : 2 * QB] = np.where(qj >= ki + P, 0.0, NEG)
    krow1 = np.ones((1, S), np.float32).astype(ml_dtypes.bfloat16)

    in_maps = []
    for j in range(8):
        b, p = j // 2, j % 2
        eb = embeddings[b].reshape(S // QB, QB, E)
        order = np.empty(S // QB, np.int64)
        for m in range(S // (2 * QB)):
            order[2 * m] = 2 * m + p
            order[2 * m + 1] = 2 * m + 1 - p
        # [NS, P, NE*SC]: chunk-major, within chunk partition-major, e-chunk
        # then position: emb[m, p_, e*SC+s] = embT[e*P + p_, m*SC + s]
        embp = np.ascontiguousarray(
            eb[order]
            .reshape(S, E)
            .astype(ml_dtypes.bfloat16)
            .T.reshape(NE, P, NS, SC)
            .transpose(2, 1, 0, 3)
            .reshape(NS, P, NE * SC)
        )
        m2 = np.full((1, NSLOT * QB), NEG if p == 0 else 0.0, np.float32).astype(
            ml_dtypes.bfloat16
        )
        in_maps.append(
            {
                "emb": embp,
                "wqk": wqk,
                "wv": wv,
                "trimask": trimask,
                "m2row": m2,
                "krow1": krow1,
            }
        )
    return in_maps


def _assemble(results):
    out = np.empty((B, S, H), np.float32)
    for j in range(8):
        b, p = j // 2, j % 2
        o = results[j]["out"]  # [16, 128, 64]
        for k in range(NSLOT):
            g0 = (2 * k + p) * QB
            out[b, g0 : g0 + P] = o[2 * k]
            out[b, g0 + P : g0 + 2 * P] = o[2 * k + 1]
    return out


def kernel(embeddings, W_Q, W_K, W_V, _trace=False, _tmpdir=None):
    if "nc" not in _CACHE:
        _CACHE["nc"] = _build_program()
    nc = _CACHE["nc"]
    in_maps = _host_inputs(
        np.asarray(embeddings), np.asarray(W_Q), np.asarray(W_K), np.asarray(W_V)
    )
    res = run_bass_kernel_spmd(
        nc, in_maps, list(range(8)), trace=_trace, tmpdir=_tmpdir
    )
    out = _assemble(res.results)
    if _trace:
        return out, res
    return out


if __name__ == "__main__":
    rng = np.random.default_rng(0)
    emb = rng.standard_normal((B, S, E), dtype=np.float32)
    wq = rng.uniform(-0.07, 0.07, (H, E)).astype(np.float32)
    wk = rng.uniform(-0.07, 0.07, (H, E)).astype(np.float32)
    wv_ = rng.uniform(-0.07, 0.07, (H, E)).astype(np.float32)
    o = kernel(emb, wq, wk, wv_)
    print("ok", o.shape, o.dtype)

